# revision 1
# baseline (speedup 1.0000x reference)
"""Trainium2 Bass kernel for nn_DCAM (dense transformer attention module).

Reference computation (per batch b):
  qp/kp/vp = avg_pool2d(feature_{q,k,v}, 2)            # (C=256, 64, 64)
  q = Wq @ qp, k = Wk @ kp  (M=32 channels)            # (32, N=4096)
  v = Wv @ vp                                          # (256, N)
  attn = softmax(q^T k, axis=-1)                       # (N, N)
  out[c, m] = sum_n v[c, n] attn[m, n]                 # (256, N)
  result = upsample_nearest(out, 2) + feature_v        # (256, 128, 128)

Sharding: data-parallel over batch B=8 across 8 NeuronCores (1 batch/core).

Per-core design notes:
  - S^T computed directly (lhsT = k j-block, rhs = q i-chunk) so the
    softmax denominator and the output matmul need no transposes.
  - All hot matmuls in bf16 (fp32r runs at ~2 cyc/row and keeps the PE
    HAM clock-gate cold). The precision-critical S path uses a hi/lo
    bf16 split: s = qh*kh + qh*kl + ql*kh (error ~2^-17).
  - S matmuls are K=32, so 4 j-blocks run concurrently in the PE array
    via tile_position row tiling (k at partition groups 0/32/64/96,
    q replicated into all four groups).
  - S psum/P tiles are paired (128, 1024) - two j-blocks side by side -
    halving ACT/DVE instruction counts. The denominator accumulators are
    also (128, 1024) (independent halves merged at i-chunk end).
  - vertical 2x2-pooling pairs are summed by the DMA itself
    (SWDGE accum_op=add); only the horizontal add runs on DVE.
  - softmax without max-subtraction (|s| <= ~15 fits fp32 easily).
  - denominator -> 1/l via DVE reciprocal on a (128, 4) view
    (DRAM-bounce transpose); broadcast back via DMA. No Ln, so a single
    ACT table set (Copy+Exp) is loaded exactly once.
  - feature_v kept resident in SBUF as bf16 for the final residual add.
  - pooling is a 2x2 *sum*; scales fold into the exp scale (1/16) and
    into WvT (x0.25) on the host.
"""
import numpy as np
import ml_dtypes

import concourse.bass as bass
import concourse.mybir as mybir
import concourse.tile as tile
from concourse import bacc
from concourse.bass_utils import run_bass_kernel_spmd

F32 = mybir.dt.float32
F32R = mybir.dt.float32r
BF16 = mybir.dt.bfloat16
AF = mybir.ActivationFunctionType
ADD = mybir.AluOpType.add

B = 8
C = 256
M = 32
H = W = 128
HP = WP = 64
N = HP * WP          # 4096
CB = C // 128        # 2 channel blocks
JB = N // 128        # 32 key blocks
JG = JB // 4         # 8 groups of 4 packed j-blocks
IC = N // 512        # 8 query chunks


def build_module():
    nc = bacc.Bacc("TRN2", target_bir_lowering=False, debug=False)

    fq_d = nc.dram_tensor("feature_q", [C, H, W], F32, kind="ExternalInput").ap()
    fk_d = nc.dram_tensor("feature_k", [C, H, W], F32, kind="ExternalInput").ap()
    fv_d = nc.dram_tensor("feature_v", [C, H, W], F32, kind="ExternalInput").ap()
    wqh_d = nc.dram_tensor("WqTh", [C, M], BF16, kind="ExternalInput").ap()
    wql_d = nc.dram_tensor("WqTl", [C, M], BF16, kind="ExternalInput").ap()
    wkh_d = nc.dram_tensor("WkTh", [C, M], BF16, kind="ExternalInput").ap()
    wkl_d = nc.dram_tensor("WkTl", [C, M], BF16, kind="ExternalInput").ap()
    wvt_d = nc.dram_tensor("WvT", [C, C], BF16, kind="ExternalInput").ap()
    out_d = nc.dram_tensor("out", [C, H, W], F32, kind="ExternalOutput").ap()

    with tile.TileContext(nc) as tc:
        with tc.tile_pool(name="const", bufs=1) as cpool, \
             tc.tile_pool(name="persist", bufs=1) as pp, \
             tc.tile_pool(name="ps", bufs=1, space="PSUM") as ps, \
             tc.tile_pool(name="dramb", bufs=2, space="DRAM") as dpool:
            # ---- constants ----
            w_sb = {}
            for nm, dram in (("qh", wqh_d), ("ql", wql_d),
                             ("kh", wkh_d), ("kl", wkl_d)):
                t = cpool.tile([128, CB, M], BF16, name=f"w_{nm}")
                nc.sync.dma_start(t[:], dram.rearrange("(b p) m -> p b m", p=128))
                w_sb[nm] = t
            wv_sb = cpool.tile([128, CB, C], BF16)
            nc.sync.dma_start(wv_sb[:], wvt_d.rearrange("(b p) c -> p b c", p=128))
            ones_col = cpool.tile([128, 1], F32R)
            nc.vector.memset(ones_col.bitcast(F32), 1.0)

            # ---- persistent tensors ----
            q4h = pp.tile([128, N], BF16)             # q hi replicated x4
            q4l = pp.tile([128, N], BF16)             # q lo replicated x4
            kh_all = pp.tile([128, JG, 128], BF16)    # [32*(jb%4)+m, jb//4, jf]
            kl_all = pp.tile([128, JG, 128], BF16)
            vt_all = pp.tile([128, JB, C], BF16)      # vT[j, c] per j-block
            fv_sb = pp.tile([128, CB, H, W], BF16)    # resident residual copy

            # fv load early: stream alongside phase A1 (casts f32->bf16)
            for cb in range(CB):
                for hh in range(2):
                    nc.gpsimd.dma_start(
                        fv_sb[:, cb, hh * 64:(hh + 1) * 64, :],
                        fv_d[cb * 128:(cb + 1) * 128,
                             hh * 64:(hh + 1) * 64, :])

            # =========== Phase A2: pool fv, project vT ===========
            with tc.tile_pool(name="poolV", bufs=1) as pv:
                for half in range(2):  # 32 pooled rows each
                    vph = pv.tile([128, CB, 32, WP], BF16, tag="vph", bufs=2,
                                  name="vph")
                    for cb in range(CB):
                        for sub in range(2):  # 16 pooled rows
                            raw0 = half * 64 + sub * 32
                            src = fv_sb[:, cb, raw0:raw0 + 32, :].rearrange(
                                "c (h dy) (w dx) -> c h dy w dx", dy=2, dx=2)
                            rfv = pv.tile([128, 16, WP, 2], BF16, tag="rfv",
                                          bufs=3, name="rfv")
                            nc.gpsimd.tensor_add(rfv[:], src[:, :, 0],
                                                 src[:, :, 1])
                            nc.gpsimd.tensor_add(
                                vph[:, cb, sub * 16:(sub + 1) * 16, :],
                                rfv[:, :, :, 0], rfv[:, :, :, 1])
                    for r2 in range(16):  # j-blocks in this half
                        jb = half * 16 + r2
                        vt_ps = ps.tile([128, 512], F32, tag="o1",
                                        bufs=2, name="vt_ps")[:, :C]
                        nc.tensor.matmul(vt_ps[:],
                                         vph[:, 0, r2 * 2:r2 * 2 + 2, :],
                                         wv_sb[:, 0], start=True, stop=False)
                        nc.tensor.matmul(vt_ps[:],
                                         vph[:, 1, r2 * 2:r2 * 2 + 2, :],
                                         wv_sb[:, 1], start=False, stop=True)
                        nc.scalar.copy(vt_all[:, jb, :], vt_ps[:])

            # =========== Phase A1: pool + project q and k ===========
            with tc.tile_pool(name="poolA", bufs=1) as pa:
                for ti, feat in ((0, fq_d), (1, fk_d)):
                    # per-tensor tags -> the fq and fk pipelines overlap;
                    # qp is a rolling per-chunk buffer, projected immediately.
                    nm = "q" if ti == 0 else "k"
                    wh = w_sb["qh" if ti == 0 else "kh"]
                    wl = w_sb["ql" if ti == 0 else "kl"]
                    for icn in range(IC):  # 8 pooled rows = one i-chunk
                        qp_h = pa.tile([128, CB, 8, WP], BF16,
                                       tag=f"qp_h{nm}", bufs=3, name="qp_h")
                        qp_l = pa.tile([128, CB, 8, WP], BF16,
                                       tag=f"qp_l{nm}", bufs=3, name="qp_l")
                        for cb in range(CB):
                            x5 = pa.tile([128, 8, 2, WP, 2], F32,
                                         tag=f"x{nm}", bufs=2, name="x5")
                            src = feat[cb * 128:(cb + 1) * 128,
                                       icn * 16:(icn + 1) * 16, :]
                            nc.sync.dma_start(
                                x5[:],
                                src.rearrange("c (h dy) (w dx) -> c h dy w dx",
                                              dy=2, dx=2))
                            r = pa.tile([128, 8, WP, 2], F32, tag=f"r{nm}",
                                        bufs=2, name="r")
                            nc.vector.tensor_add(r[:], x5[:, :, 0], x5[:, :, 1])
                            qpc = pa.tile([128, 8, WP], F32, tag=f"qpc{nm}",
                                          bufs=2, name="qpc")
                            nc.vector.tensor_add(qpc[:], r[:, :, :, 0],
                                                 r[:, :, :, 1])
                            nc.scalar.copy(qp_h[:, cb], qpc[:])
                            nc.vector.tensor_sub(qp_l[:, cb], qpc[:],
                                                 qp_h[:, cb])
                        pr_ps = ps.tile([128, 512], F32,
                                        tag="o0" if ti == 0 else "o1",
                                        bufs=2, name="pr_ps")[:M, :]
                        mms = [(wt, qt, cb)
                               for cb in range(CB)
                               for (wt, qt) in ((wh, qp_h), (wh, qp_l),
                                                (wl, qp_h))]
                        for mi, (wt, qt, cb) in enumerate(mms):
                            nc.tensor.matmul(pr_ps[:], wt[:, cb],
                                             qt[:, cb],
                                             start=(mi == 0),
                                             stop=(mi == len(mms) - 1),
                                             skip_group_check=True)
                        # evict hi/lo; k goes into the packed layout
                        if ti == 0:
                            cs = slice(icn * 512, (icn + 1) * 512)
                            nc.scalar.copy(q4h[0:32, cs], pr_ps[:])
                            nc.vector.tensor_sub(q4l[0:32, cs], pr_ps[:],
                                                 q4h[0:32, cs])
                            # replicate this chunk into partition groups
                            # 1..3 immediately, so attention for this
                            # i-chunk can start without waiting for all
                            # of feature_q.
                            for g in range(1, 4):
                                gs = slice(g * 32, (g + 1) * 32)
                                nc.sync.dma_start(q4h[gs, cs],
                                                 q4h[0:32, cs])
                                nc.sync.dma_start(q4l[gs, cs],
                                                 q4l[0:32, cs])
                        else:
                            for t in range(4):
                                pslc = pr_ps[:, t * 128:(t + 1) * 128]
                                nc.scalar.copy(kh_all[t * 32:(t + 1) * 32,
                                                      icn, :], pslc)
                                nc.vector.tensor_sub(
                                    kl_all[t * 32:(t + 1) * 32, icn, :],
                                    pslc, kh_all[t * 32:(t + 1) * 32, icn, :])

            # =========== Phase B: attention + fused epilogue ===========
            TERMS = ((0, 0), (0, 1), (1, 0))  # (k hi/lo, q hi/lo)
            with tc.tile_pool(name="poolB", bufs=1) as pb:
                for ic in range(IC):
                    i0 = ic * 512
                    lacc_d = pb.tile([128, 1024], F32R, tag="lacc_d", bufs=2,
                                     name="lacc_d")
                    lacc_g = pb.tile([128, 1024], F32R, tag="lacc_g", bufs=2,
                                     name="lacc_g")
                    nc.vector.memset(lacc_d.bitcast(F32), 0.0)
                    nc.gpsimd.memset(lacc_g.bitcast(F32), 0.0)
                    o_ps = [ps.tile([128, 512], F32, tag=f"o{cb}", bufs=2,
                                     name=f"o{cb}_ps")
                            for cb in range(CB)]
                    for jg in range(JG):
                        s_ps = [ps.tile([128, 1024], F32, tag=f"s{u}",
                                         bufs=1, name=f"s{u}_ps")
                                for u in range(2)]
                        for t in range(4):
                            gs = slice(t * 32, (t + 1) * 32)
                            dst = s_ps[t // 2][:, (t % 2) * 512:
                                               (t % 2) * 512 + 512]
                            for term, (kk, qq) in enumerate(TERMS):
                                ka = kh_all if kk == 0 else kl_all
                                qa = q4h if qq == 0 else q4l
                                nc.tensor.matmul(
                                    dst, ka[gs, jg, :], qa[gs, i0:i0 + 512],
                                    start=(term == 0), stop=(term == 2),
                                    tile_position=(t * 32, 0),
                                    skip_group_check=True)
                        p_t = []
                        for u in range(2):
                            p = pb.tile([128, 1024], BF16, tag="p", bufs=8,
                                        name="p")
                            nc.scalar.activation(p[:], s_ps[u][:], AF.Exp,
                                                 scale=0.0625)
                            p_t.append(p)
                        nc.vector.tensor_add(lacc_d[:], lacc_d[:], p_t[0][:])
                        nc.gpsimd.tensor_add(lacc_g[:], lacc_g[:], p_t[1][:])
                        for u in range(2):
                            for tt in range(2):
                                j = jg * 4 + u * 2 + tt
                                pr = p_t[u][:, tt * 512:tt * 512 + 512]
                                for cb in range(CB):
                                    nc.tensor.matmul(
                                        o_ps[cb][:],
                                        vt_all[:, j, cb * 128:(cb + 1) * 128],
                                        pr,
                                        start=(j == 0), stop=(j == JB - 1),
                                        skip_group_check=True)
                    # ---- fused epilogue for this i-chunk ----
                    # l = column sums of all four accumulator halves, merged
                    # for free by PSUM accumulation across four ones-matmuls.
                    l_ps = ps.tile([128, 1024], F32, tag="s0", bufs=1,
                                    name="l_ps")
                    halves = [lacc_d[:, :512], lacc_d[:, 512:],
                              lacc_g[:, :512], lacc_g[:, 512:]]
                    for hi_, hv in enumerate(halves):
                        nc.tensor.matmul(l_ps[:1, :512], ones_col[:], hv,
                                         start=(hi_ == 0),
                                         stop=(hi_ == len(halves) - 1),
                                         skip_group_check=True)
                    l_sb = pb.tile([1, 512], F32, tag="l_sb", bufs=2,
                                   name="l_sb")
                    nc.scalar.copy(l_sb[:], l_ps[:1, :512])
                    # transpose to (128, 4) via DRAM bounce, reciprocal, back
                    l_dr = dpool.tile([512], F32, tag="l_dr", bufs=2,
                                      name="l_dr")
                    nc.sync.dma_start(l_dr[:], l_sb[:])
                    lT = pb.tile([128, 4], F32, tag="lT", bufs=2, name="lT")
                    nc.sync.dma_start(lT[:], l_dr.rearrange("(p b) -> p b",
                                                            b=4))
                    rT = pb.tile([128, 4], F32, tag="rT", bufs=2, name="rT")
                    nc.vector.reciprocal(rT[:], lT[:])
                    r_dr = dpool.tile([512], F32, tag="r_dr", bufs=2,
                                      name="r_dr")
                    nc.sync.dma_start(r_dr.rearrange("(p b) -> p b", b=4),
                                      rT[:])
                    rb_sb = pb.tile([128, 512], F32, tag="rb_sb", bufs=2,
                                    name="rb_sb")
                    nc.sync.dma_start(
                        rb_sb[:],
                        r_dr.rearrange("(o x) -> o x", o=1).to_broadcast(
                            (128, 512)))
                    for cb in range(CB):
                        oc = pb.tile([128, 512], F32, tag="oc", bufs=4,
                                     name="oc")
                        nc.vector.tensor_mul(oc[:], o_ps[cb][:], rb_sb[:])
                        final = pb.tile([128, 8, 2, WP, 2], F32, tag="final",
                                        bufs=3, name="final")
                        up = oc.rearrange("c (h w) -> c h w", w=WP)[
                            :, :, :, None].to_broadcast((128, 8, WP, 2))
                        fvv = fv_sb[:, cb, ic * 16:(ic + 1) * 16, :].rearrange(
                            "c (h dy) (w dx) -> c h dy w dx", dy=2, dx=2)
                        nc.vector.tensor_add(final[:, :, 0], up, fvv[:, :, 0])
                        nc.vector.tensor_add(final[:, :, 1], up, fvv[:, :, 1])
                        nc.sync.dma_start(
                            out_d[cb * 128:(cb + 1) * 128,
                                  ic * 16:(ic + 1) * 16, :],
                            final.rearrange("c h dy w dx -> c (h dy) (w dx)"))

    nc.compile()
    return nc


_NC_CACHE = []
LAST_RESULT = []  # last BassKernelResults, for perf inspection by test.py


def _bf16_split(x):
    hi = x.astype(ml_dtypes.bfloat16)
    lo = (x - hi.astype(np.float32)).astype(ml_dtypes.bfloat16)
    return np.ascontiguousarray(hi), np.ascontiguousarray(lo)


def kernel(**inputs) -> np.ndarray:
    fq = np.ascontiguousarray(np.asarray(inputs["feature_q"], dtype=np.float32))
    fk = np.ascontiguousarray(np.asarray(inputs["feature_k"], dtype=np.float32))
    fv = np.ascontiguousarray(np.asarray(inputs["feature_v"], dtype=np.float32))
    wq = np.asarray(inputs["Wq"], dtype=np.float32)
    wk = np.asarray(inputs["Wk"], dtype=np.float32)
    wv = np.asarray(inputs["Wv"], dtype=np.float32)

    # weight layout prep (pure layout/scale folding, no heavy compute):
    # on-device pooling is a 2x2 *sum*; q,k each pick up 4x -> s is 16x,
    # folded into the on-device exp scale; v's 4x is folded into WvT here.
    wqh, wql = _bf16_split(wq.T)                      # (C, M) hi/lo
    wkh, wkl = _bf16_split(wk.T)
    wvt = np.ascontiguousarray(
        (wv.T * 0.25).astype(ml_dtypes.bfloat16))     # (C, C) [c_in, c_out]

    if not _NC_CACHE:
        _NC_CACHE.append(build_module())
    nc = _NC_CACHE[0]

    in_maps = [
        {
            "feature_q": fq[b],
            "feature_k": fk[b],
            "feature_v": fv[b],
            "WqTh": wqh,
            "WqTl": wql,
            "WkTh": wkh,
            "WkTl": wkl,
            "WvT": wvt,
        }
        for b in range(B)
    ]
    res = run_bass_kernel_spmd(nc, in_maps, core_ids=list(range(B)))
    LAST_RESULT.clear()
    LAST_RESULT.append(res)
    out = np.stack([res.results[b]["out"] for b in range(B)], axis=0)
    return out.astype(np.float32)


if __name__ == "__main__":
    nc = build_module()
    print("module built + compiled OK")



# revision 2
# speedup vs baseline: 1.0444x; 1.0444x over previous
"""Trainium2 Bass kernel for nn_DCAM (dense transformer attention module).

Reference computation (per batch b):
  qp/kp/vp = avg_pool2d(feature_{q,k,v}, 2)            # (C=256, 64, 64)
  q = Wq @ qp, k = Wk @ kp  (M=32 channels)            # (32, N=4096)
  v = Wv @ vp                                          # (256, N)
  attn = softmax(q^T k, axis=-1)                       # (N, N)
  out[c, m] = sum_n v[c, n] attn[m, n]                 # (256, N)
  result = upsample_nearest(out, 2) + feature_v        # (256, 128, 128)

Sharding: data-parallel over batch B=8 across 8 NeuronCores (1 batch/core).

Per-core design (v2 — restructured from the hi/lo baseline):
  - All feature inputs are pre-cast to bf16 on the host; output is written
    bf16 and upcast on the host. Halves all HBM traffic.
  - q/k single bf16 (no hi/lo split): 1 S-term instead of 3. The 2e-2
    rel-err budget has ~7x slack over this.
  - The entire 2x2 sum-pooling of q/k is folded into the projection
    matmuls: 8 accumulating MMs per chunk with strided rhs APs
    (dy/dx slices of the raw 16x128 row block). No pooling DVE work at
    all on the q/k path.
  - v pooling stays a 2-step gpsimd add (from the resident fv copy);
    projection per j-block as before.
  - Phase order: fv+fk stream first (separate DMA queues) with V-pool/
    V-proj and K-proj interleaved; fq streams last and Phase B chases it
    per i-chunk, overlapping the attention with the tail of input DMA.
  - Phase B per jg: one [128,2048] S psum (4 j-blocks x 512 i), a single
    [128,2048] exp ACTIVATE (ACT does exp ONLY; all copies/evictions are
    on DVE), O-MMs per j-block/cb, and a bf16 DVE running sum for the
    softmax denominator (merged by a ones-matmul at i-chunk end).
  - softmax without max-subtraction (|s| <= ~15 fits f32/bf16 easily).
  - pooling is a 2x2 *sum*; scales fold into the exp scale (1/16) and
    into WvT (x0.25) on the host.
"""
import numpy as np
import ml_dtypes

import concourse.bass as bass
import concourse.mybir as mybir
import concourse.tile as tile
from concourse import bacc
from concourse.bass_utils import run_bass_kernel_spmd

F32 = mybir.dt.float32
BF16 = mybir.dt.bfloat16
AF = mybir.ActivationFunctionType

B = 8
C = 256
M = 32
H = W = 128
HP = WP = 64
N = HP * WP          # 4096
CB = C // 128        # 2 channel blocks
JB = N // 128        # 32 key blocks
JG = JB // 4         # 8 groups of 4 packed j-blocks
IC = N // 512        # 8 query chunks of 512


def build_module():
    nc = bacc.Bacc("TRN2", target_bir_lowering=False, debug=False)

    fq_d = nc.dram_tensor("feature_q", [C, H, W], BF16, kind="ExternalInput").ap()
    fk_d = nc.dram_tensor("feature_k", [C, H, W], BF16, kind="ExternalInput").ap()
    fv_d = nc.dram_tensor("feature_v", [C, H, W], BF16, kind="ExternalInput").ap()
    wqt_d = nc.dram_tensor("WqT", [C, M], BF16, kind="ExternalInput").ap()
    wkt_d = nc.dram_tensor("WkT", [C, M], BF16, kind="ExternalInput").ap()
    wvt_d = nc.dram_tensor("WvT", [C, C], BF16, kind="ExternalInput").ap()
    out_d = nc.dram_tensor("out", [C, H, W], BF16, kind="ExternalOutput").ap()

    with tile.TileContext(nc) as tc:
        with tc.tile_pool(name="const", bufs=1) as cpool, \
             tc.tile_pool(name="persist", bufs=1) as pp, \
             tc.tile_pool(name="ps", bufs=1, space="PSUM") as ps, \
             tc.tile_pool(name="dramb", bufs=2, space="DRAM") as dpool:
            # ---- constants ----
            wq_sb = cpool.tile([128, CB, M], BF16, name="wq")
            nc.sync.dma_start(wq_sb[:], wqt_d.rearrange("(b p) m -> p b m", p=128))
            wk_sb = cpool.tile([128, CB, M], BF16, name="wk")
            nc.sync.dma_start(wk_sb[:], wkt_d.rearrange("(b p) m -> p b m", p=128))
            wv_sb = cpool.tile([128, CB, C], BF16)
            nc.sync.dma_start(wv_sb[:], wvt_d.rearrange("(b p) c -> p b c", p=128))
            ones_b = cpool.tile([128, 1], BF16)
            nc.vector.memset(ones_b[:], 1.0)

            # ---- persistent tensors ----
            q4 = pp.tile([128, N], BF16)              # q replicated x4 groups
            k_all = pp.tile([128, JG, 128], BF16)     # [32*(jb%4)+m, jb//4, jf]
            vt_all = pp.tile([128, JB, C], BF16)      # vT[j, c] per j-block
            fv_sb = pp.tile([128, CB, H, W], BF16)    # resident residual copy

            # =========== Phase A: stream fv + fk, pool/project ===========
            # fv slabs on the gpsimd DMA queue, fk chunks on the sync queue:
            # the two streams share HBM BW and their compute interleaves.
            def k_chunk(icn, feat, w_sb, is_q):
                xs = []
                for cb in range(CB):
                    x = pa.tile([128, 16, W], BF16, tag=f"x{'q' if is_q else 'k'}",
                                bufs=3, name="x")
                    nc.sync.dma_start(
                        x[:], feat[cb * 128:(cb + 1) * 128,
                                   icn * 16:(icn + 1) * 16, :])
                    xs.append(x)
                pr_ps = ps.tile([128, 512], F32, tag="a", bufs=2,
                                name="pr_ps")[:M, :]
                mms = [(cb, dy, dx) for cb in range(CB)
                       for dy in range(2) for dx in range(2)]
                for mi, (cb, dy, dx) in enumerate(mms):
                    rhs = xs[cb].rearrange("c (h dy) (w dx) -> c h dy w dx",
                                           dy=2, dx=2)[:, :, dy, :, dx]
                    nc.tensor.matmul(pr_ps[:], w_sb[:, cb], rhs,
                                     start=(mi == 0), stop=(mi == len(mms) - 1),
                                     skip_group_check=True)
                cs = slice(icn * 512, (icn + 1) * 512)
                if is_q:
                    nc.vector.tensor_scalar_add(q4[0:32, cs], pr_ps[:], 0.0)
                    for g in range(1, 4):
                        nc.sync.dma_start(q4[g * 32:(g + 1) * 32, cs],
                                          q4[0:32, cs])
                else:
                    for t in range(4):
                        nc.vector.tensor_scalar_add(
                            k_all[t * 32:(t + 1) * 32, icn, :],
                            pr_ps[:, t * 128:(t + 1) * 128], 0.0)

            with tc.tile_pool(name="poolA", bufs=1) as pa:
                for slab in range(4):   # 32 raw rows -> 16 pooled rows each
                    r0 = slab * 32
                    for cb in range(CB):
                        nc.gpsimd.dma_start(
                            fv_sb[:, cb, r0:r0 + 32, :],
                            fv_d[cb * 128:(cb + 1) * 128, r0:r0 + 32, :])
                    vph = pa.tile([128, CB, 16, WP], BF16, tag="vph", bufs=2,
                                  name="vph")
                    for cb in range(CB):
                        src = fv_sb[:, cb, r0:r0 + 32, :].rearrange(
                            "c (h dy) (w dx) -> c h dy w dx", dy=2, dx=2)
                        rfv = pa.tile([128, 16, WP, 2], BF16, tag="rfv",
                                      bufs=2, name="rfv")
                        nc.gpsimd.tensor_add(rfv[:], src[:, :, 0], src[:, :, 1])
                        nc.gpsimd.tensor_add(vph[:, cb], rfv[:, :, :, 0],
                                             rfv[:, :, :, 1])
                    for r2 in range(8):   # j-blocks in this slab
                        jb = slab * 8 + r2
                        vt_ps = ps.tile([128, 512], F32, tag="a",
                                        bufs=2, name="vt_ps")[:, :C]
                        nc.tensor.matmul(vt_ps[:],
                                         vph[:, 0, r2 * 2:r2 * 2 + 2, :],
                                         wv_sb[:, 0], start=True, stop=False)
                        nc.tensor.matmul(vt_ps[:],
                                         vph[:, 1, r2 * 2:r2 * 2 + 2, :],
                                         wv_sb[:, 1], start=False, stop=True)
                        nc.vector.tensor_scalar_add(vt_all[:, jb, :],
                                                    vt_ps[:], 0.0)
                    for icn in (slab * 2, slab * 2 + 1):
                        k_chunk(icn, fk_d, wk_sb, is_q=False)
                for icn in range(IC):
                    k_chunk(icn, fq_d, wq_sb, is_q=True)

            # =========== Phase B: attention + fused epilogue ===========
            with tc.tile_pool(name="poolB", bufs=1) as pb:
                for ic in range(IC):
                    i0 = ic * 512
                    lacc = pb.tile([128, 2048], BF16, tag="lacc", bufs=2,
                                   name="lacc")
                    o_ps = [ps.tile([128, 512], F32, tag=f"o{cb}", bufs=1,
                                    name=f"o{cb}_ps")
                            for cb in range(CB)]
                    for jg in range(JG):
                        s_ps = ps.tile([128, 2048], F32, tag="s", bufs=1,
                                       name="s_ps")
                        for t in range(4):
                            gs = slice(t * 32, (t + 1) * 32)
                            nc.tensor.matmul(
                                s_ps[:, t * 512:(t + 1) * 512],
                                k_all[gs, jg, :], q4[gs, i0:i0 + 512],
                                start=True, stop=True,
                                tile_position=(t * 32, 0),
                                skip_group_check=True)
                        p = pb.tile([128, 2048], BF16, tag="p", bufs=4,
                                    name="p")
                        nc.scalar.activation(p[:], s_ps[:], AF.Exp,
                                             scale=0.0625)
                        if jg == 0:
                            nc.vector.tensor_scalar_add(lacc[:], p[:], 0.0)
                        else:
                            nc.vector.tensor_add(lacc[:], lacc[:], p[:])
                        for t in range(4):
                            j = jg * 4 + t
                            pr = p[:, t * 512:(t + 1) * 512]
                            for cb in range(CB):
                                nc.tensor.matmul(
                                    o_ps[cb][:],
                                    vt_all[:, j, cb * 128:(cb + 1) * 128],
                                    pr,
                                    start=(j == 0), stop=(j == JB - 1),
                                    skip_group_check=True)
                    # ---- fused epilogue for this i-chunk ----
                    l_ps = ps.tile([128, 2048], F32, tag="s", bufs=1,
                                   name="l_ps")
                    for t in range(4):
                        nc.tensor.matmul(l_ps[:1, :512], ones_b[:],
                                         lacc[:, t * 512:(t + 1) * 512],
                                         start=(t == 0), stop=(t == 3),
                                         skip_group_check=True)
                    rb1 = pb.tile([1, 512], F32, tag="rb1", bufs=2,
                                  name="rb1")
                    nc.vector.reciprocal(rb1[:], l_ps[:1, :512])
                    r_dr = dpool.tile([512], F32, tag="r_dr", bufs=2,
                                      name="r_dr")
                    nc.sync.dma_start(r_dr[:], rb1[:])
                    rb = pb.tile([128, 512], F32, tag="rb", bufs=2,
                                 name="rb")
                    nc.sync.dma_start(
                        rb[:],
                        r_dr.rearrange("(o x) -> o x", o=1).to_broadcast(
                            (128, 512)))
                    for cb in range(CB):
                        oc = pb.tile([128, 512], BF16, tag="oc", bufs=2,
                                     name="oc")
                        nc.vector.tensor_mul(oc[:], o_ps[cb][:], rb[:])
                        final = pb.tile([128, 8, 2, WP, 2], BF16, tag="final",
                                        bufs=3, name="final")
                        up = oc.rearrange("c (h w) -> c h w", w=WP)[
                            :, :, :, None].to_broadcast((128, 8, WP, 2))
                        fvv = fv_sb[:, cb, ic * 16:(ic + 1) * 16, :].rearrange(
                            "c (h dy) (w dx) -> c h dy w dx", dy=2, dx=2)
                        nc.vector.tensor_add(final[:, :, 0], up, fvv[:, :, 0])
                        nc.vector.tensor_add(final[:, :, 1], up, fvv[:, :, 1])
                        nc.sync.dma_start(
                            out_d[cb * 128:(cb + 1) * 128,
                                  ic * 16:(ic + 1) * 16, :],
                            final.rearrange("c h dy w dx -> c (h dy) (w dx)"))

    nc.compile()
    return nc


_NC_CACHE = []
LAST_RESULT = []  # last BassKernelResults, for perf inspection by test.py


def kernel(**inputs) -> np.ndarray:
    fq = np.ascontiguousarray(
        np.asarray(inputs["feature_q"], dtype=np.float32).astype(
            ml_dtypes.bfloat16))
    fk = np.ascontiguousarray(
        np.asarray(inputs["feature_k"], dtype=np.float32).astype(
            ml_dtypes.bfloat16))
    fv = np.ascontiguousarray(
        np.asarray(inputs["feature_v"], dtype=np.float32).astype(
            ml_dtypes.bfloat16))
    wq = np.asarray(inputs["Wq"], dtype=np.float32)
    wk = np.asarray(inputs["Wk"], dtype=np.float32)
    wv = np.asarray(inputs["Wv"], dtype=np.float32)

    # weight layout prep (pure layout/scale folding, no heavy compute):
    # on-device pooling is a 2x2 *sum*; q,k each pick up 4x -> s is 16x,
    # folded into the on-device exp scale; v's 4x is folded into WvT here.
    wqt = np.ascontiguousarray(wq.T.astype(ml_dtypes.bfloat16))
    wkt = np.ascontiguousarray(wk.T.astype(ml_dtypes.bfloat16))
    wvt = np.ascontiguousarray(
        (wv.T * 0.25).astype(ml_dtypes.bfloat16))     # (C, C) [c_in, c_out]

    if not _NC_CACHE:
        _NC_CACHE.append(build_module())
    nc = _NC_CACHE[0]

    in_maps = [
        {
            "feature_q": fq[b],
            "feature_k": fk[b],
            "feature_v": fv[b],
            "WqT": wqt,
            "WkT": wkt,
            "WvT": wvt,
        }
        for b in range(B)
    ]
    res = run_bass_kernel_spmd(nc, in_maps, core_ids=list(range(B)))
    LAST_RESULT.clear()
    LAST_RESULT.append(res)
    out = np.stack([np.asarray(res.results[b]["out"]) for b in range(B)],
                   axis=0)
    return out.astype(np.float32)


if __name__ == "__main__":
    nc = build_module()
    print("module built + compiled OK")


# revision 5
# speedup vs baseline: 1.6701x; 1.5991x over previous
"""Trainium2 Bass kernel for nn_DCAM (dense transformer attention module).

Reference computation (per batch b):
  qp/kp/vp = avg_pool2d(feature_{q,k,v}, 2)            # (C=256, 64, 64)
  q = Wq @ qp, k = Wk @ kp  (M=32 channels)            # (32, N=4096)
  v = Wv @ vp                                          # (256, N)
  attn = softmax(q^T k, axis=-1)                       # (N, N)
  out[c, m] = sum_n v[c, n] attn[m, n]                 # (256, N)
  result = upsample_nearest(out, 2) + feature_v        # (256, 128, 128)

Sharding: data-parallel over batch B=8 across 8 NeuronCores (1 batch/core).

Per-core design (v2 — restructured from the hi/lo baseline):
  - All feature inputs are pre-cast to bf16 on the host; output is written
    bf16 and upcast on the host. Halves all HBM traffic.
  - q/k single bf16 (no hi/lo split): 1 S-term instead of 3. The 2e-2
    rel-err budget has ~7x slack over this.
  - The entire 2x2 sum-pooling of q/k is folded into the projection
    matmuls: 8 accumulating MMs per chunk with strided rhs APs
    (dy/dx slices of the raw 16x128 row block). No pooling DVE work at
    all on the q/k path.
  - v pooling stays a 2-step gpsimd add (from the resident fv copy);
    projection per j-block as before.
  - Phase order: fv+fk stream first (separate DMA queues) with V-pool/
    V-proj and K-proj interleaved; fq streams last and Phase B chases it
    per i-chunk, overlapping the attention with the tail of input DMA.
  - Phase B per jg: one [128,2048] S psum (4 j-blocks x 512 i), a single
    [128,2048] exp ACTIVATE (ACT does exp ONLY; all copies/evictions are
    on DVE), O-MMs per j-block/cb, and a bf16 DVE running sum for the
    softmax denominator (merged by a ones-matmul at i-chunk end).
  - softmax without max-subtraction (|s| <= ~15 fits f32/bf16 easily).
  - pooling is a 2x2 *sum*; scales fold into the exp scale (1/16) and
    into WvT (x0.25) on the host.
"""
import numpy as np
import ml_dtypes

import concourse.bass as bass
import concourse.mybir as mybir
import concourse.tile as tile
from concourse import bacc
from concourse.bass_utils import run_bass_kernel_spmd

F32 = mybir.dt.float32
BF16 = mybir.dt.bfloat16
AF = mybir.ActivationFunctionType

B = 8
C = 256
M = 32
H = W = 128
HP = WP = 64
N = HP * WP          # 4096
CB = C // 128        # 2 channel blocks
JB = N // 128        # 32 key blocks
JG = JB // 4         # 8 groups of 4 packed j-blocks
IC = N // 512        # 8 query chunks of 512


def build_module():
    nc = bacc.Bacc("TRN2", target_bir_lowering=False, debug=False)

    fq_d = nc.dram_tensor("feature_q", [C, H, W], BF16, kind="ExternalInput").ap()
    fk_d = nc.dram_tensor("feature_k", [C, H, W], BF16, kind="ExternalInput").ap()
    fv_d = nc.dram_tensor("feature_v", [C, H, W], BF16, kind="ExternalInput").ap()
    wqt_d = nc.dram_tensor("WqT", [C, M], BF16, kind="ExternalInput").ap()
    wkt_d = nc.dram_tensor("WkT", [C, M], BF16, kind="ExternalInput").ap()
    wvt_d = nc.dram_tensor("WvT", [C, C], BF16, kind="ExternalInput").ap()
    out_d = nc.dram_tensor("out", [C, H, W], BF16, kind="ExternalOutput").ap()

    with tile.TileContext(nc) as tc:
        with tc.tile_pool(name="const", bufs=1) as cpool, \
             tc.tile_pool(name="persist", bufs=1) as pp, \
             tc.tile_pool(name="ps", bufs=1, space="PSUM") as ps, \
             tc.tile_pool(name="dramb", bufs=2, space="DRAM") as dpool:
            # ---- constants ----
            wq_sb = cpool.tile([128, CB, M], BF16, name="wq")
            nc.sync.dma_start(wq_sb[:], wqt_d.rearrange("(b p) m -> p b m", p=128))
            wk_sb = cpool.tile([128, CB, M], BF16, name="wk")
            nc.sync.dma_start(wk_sb[:], wkt_d.rearrange("(b p) m -> p b m", p=128))
            wv_sb = cpool.tile([128, CB, C], BF16)
            nc.sync.dma_start(wv_sb[:], wvt_d.rearrange("(b p) c -> p b c", p=128))
            ones_b = cpool.tile([128, 1], BF16)
            nc.vector.memset(ones_b[:], 1.0)

            # ---- persistent tensors ----
            q4 = pp.tile([128, N], BF16)              # q replicated x4 groups
            k_all = pp.tile([128, JG, 128], BF16)     # [32*(jb%4)+m, jb//4, jf]
            vt_all = pp.tile([128, JB, C], BF16)      # vT[j, c] per j-block
            fv_sb = pp.tile([128, CB, H, W], BF16)    # resident residual copy

            # =========== Phase A: stream fv + fk, pool/project ===========
            # fv slabs on the gpsimd DMA queue, fk chunks on the sync queue:
            # the two streams share HBM BW and their compute interleaves.
            def k_chunk(icn, feat, w_sb, is_q):
                xs = []
                for cb in range(CB):
                    x = pa.tile([128, 16, W], BF16, tag=f"x{'q' if is_q else 'k'}",
                                bufs=3, name="x")
                    nc.sync.dma_start(
                        x[:], feat[cb * 128:(cb + 1) * 128,
                                   icn * 16:(icn + 1) * 16, :])
                    xs.append(x)
                pr_ps = ps.tile([128, 512], F32, tag="a", bufs=2,
                                name="pr_ps")[:M, :]
                mms = [(cb, dy, dx) for cb in range(CB)
                       for dy in range(2) for dx in range(2)]
                for mi, (cb, dy, dx) in enumerate(mms):
                    rhs = xs[cb].rearrange("c (h dy) (w dx) -> c h dy w dx",
                                           dy=2, dx=2)[:, :, dy, :, dx]
                    nc.tensor.matmul(pr_ps[:], w_sb[:, cb], rhs,
                                     start=(mi == 0), stop=(mi == len(mms) - 1),
                                     skip_group_check=True)
                cs = slice(icn * 512, (icn + 1) * 512)
                if is_q:
                    nc.vector.tensor_scalar_add(q4[0:32, cs], pr_ps[:], 0.0)
                    for g in range(1, 4):
                        nc.sync.dma_start(q4[g * 32:(g + 1) * 32, cs],
                                          q4[0:32, cs])
                else:
                    for t in range(4):
                        nc.vector.tensor_scalar_add(
                            k_all[t * 32:(t + 1) * 32, icn, :],
                            pr_ps[:, t * 128:(t + 1) * 128], 0.0)

            with tc.tile_pool(name="poolA", bufs=1) as pa:
                for slab in range(4):   # 32 raw rows -> 16 pooled rows each
                    r0 = slab * 32
                    for cb in range(CB):
                        nc.gpsimd.dma_start(
                            fv_sb[:, cb, r0:r0 + 32, :],
                            fv_d[cb * 128:(cb + 1) * 128, r0:r0 + 32, :])
                    vph = pa.tile([128, CB, 16, WP], BF16, tag="vph", bufs=2,
                                  name="vph")
                    for cb in range(CB):
                        src = fv_sb[:, cb, r0:r0 + 32, :].rearrange(
                            "c (h dy) (w dx) -> c h dy w dx", dy=2, dx=2)
                        rfv = pa.tile([128, 16, WP, 2], BF16, tag="rfv",
                                      bufs=2, name="rfv")
                        nc.vector.tensor_add(rfv[:], src[:, :, 0], src[:, :, 1])
                        nc.gpsimd.tensor_add(vph[:, cb], rfv[:, :, :, 0],
                                             rfv[:, :, :, 1])
                    for r2 in range(8):   # j-blocks in this slab
                        jb = slab * 8 + r2
                        vt_ps = ps.tile([128, 512], F32, tag="a",
                                        bufs=2, name="vt_ps")[:, :C]
                        nc.tensor.matmul(vt_ps[:],
                                         vph[:, 0, r2 * 2:r2 * 2 + 2, :],
                                         wv_sb[:, 0], start=True, stop=False)
                        nc.tensor.matmul(vt_ps[:],
                                         vph[:, 1, r2 * 2:r2 * 2 + 2, :],
                                         wv_sb[:, 1], start=False, stop=True)
                        nc.vector.tensor_scalar_add(vt_all[:, jb, :],
                                                    vt_ps[:], 0.0)
                    for icn in (slab * 2, slab * 2 + 1):
                        k_chunk(icn, fk_d, wk_sb, is_q=False)
                for icn in range(IC):
                    k_chunk(icn, fq_d, wq_sb, is_q=True)

            # =========== Phase B: attention + fused epilogue ===========
            # Software-pipelined by one jg: the O matmuls for jg-1 are
            # issued after exp(jg), so the PE streams O(jg-1) while ACT
            # computes exp(jg) - neither engine waits on the other inside
            # the steady state, and the PE stays HAM-warm.
            with tc.tile_pool(name="poolB", bufs=1) as pb:
                def o_mms(ic, jg, p):
                    for t in range(4):
                        j = jg * 4 + t
                        pr = p[:, t * 512:(t + 1) * 512]
                        for cb in range(CB):
                            nc.tensor.matmul(
                                o_ps[cb][:],
                                vt_all[:, j, cb * 128:(cb + 1) * 128],
                                pr,
                                start=(j == 0), stop=(j == JB - 1),
                                skip_group_check=True)

                for ic in range(IC):
                    i0 = ic * 512
                    lacc = pb.tile([128, 2048], BF16, tag="lacc", bufs=2,
                                   name="lacc")
                    o_ps = [ps.tile([128, 512], F32, tag=f"o{cb}", bufs=1,
                                    name=f"o{cb}_ps")
                            for cb in range(CB)]
                    p_prev = None
                    for jg in range(JG):
                        s_ps = ps.tile([128, 2048], F32, tag="s", bufs=1,
                                       name="s_ps")
                        for t in range(4):
                            gs = slice(t * 32, (t + 1) * 32)
                            nc.tensor.matmul(
                                s_ps[:, t * 512:(t + 1) * 512],
                                k_all[gs, jg, :], q4[gs, i0:i0 + 512],
                                start=True, stop=True,
                                tile_position=(t * 32, 0),
                                skip_group_check=True)
                        p = pb.tile([128, 2048], BF16, tag="p", bufs=4,
                                    name="p")
                        nc.scalar.activation(p[:], s_ps[:], AF.Exp,
                                             scale=0.0625)
                        if p_prev is not None:
                            o_mms(ic, jg - 1, p_prev)
                        if jg == 0:
                            nc.vector.tensor_scalar_add(lacc[:], p[:], 0.0)
                        else:
                            nc.vector.tensor_add(lacc[:], lacc[:], p[:])
                        p_prev = p
                    o_mms(ic, JG - 1, p_prev)
                    # ---- epilogue: fully async off the jg pipeline ----
                    # evict o psum to SBUF immediately (frees the banks for
                    # the next i-chunk); the softmax 1/l normalization and
                    # the upsample+residual run from SBUF behind the scenes.
                    o_sb = []
                    for cb in range(CB):
                        t_ = pb.tile([128, 512], F32, tag=f"osb{cb}", bufs=2,
                                     name=f"osb{cb}")
                        nc.vector.tensor_scalar_add(t_[:], o_ps[cb][:], 0.0)
                        o_sb.append(t_)
                    l_ps = ps.tile([128, 2048], F32, tag="s", bufs=1,
                                   name="l_ps")
                    for t in range(4):
                        nc.tensor.matmul(l_ps[:1, :512], ones_b[:],
                                         lacc[:, t * 512:(t + 1) * 512],
                                         start=(t == 0), stop=(t == 3),
                                         skip_group_check=True)
                    # 1/l on a (128, 4) transposed view (DRAM bounce), then
                    # broadcast back across partitions.
                    l_sb = pb.tile([1, 512], F32, tag="l_sb", bufs=2,
                                   name="l_sb")
                    nc.vector.tensor_scalar_add(l_sb[:], l_ps[:1, :512], 0.0)
                    l_dr = dpool.tile([512], F32, tag="l_dr", bufs=2,
                                      name="l_dr")
                    nc.sync.dma_start(l_dr[:], l_sb[:])
                    lT = pb.tile([128, 4], F32, tag="lT", bufs=2, name="lT")
                    nc.sync.dma_start(lT[:], l_dr.rearrange("(p b) -> p b",
                                                            b=4))
                    rT = pb.tile([128, 4], F32, tag="rT", bufs=2, name="rT")
                    nc.vector.reciprocal(rT[:], lT[:])
                    r_dr = dpool.tile([512], F32, tag="r_dr", bufs=2,
                                      name="r_dr")
                    nc.sync.dma_start(r_dr.rearrange("(p b) -> p b", b=4),
                                      rT[:])
                    rb = pb.tile([128, 512], F32, tag="rb", bufs=2,
                                 name="rb")
                    nc.sync.dma_start(
                        rb[:],
                        r_dr.rearrange("(o x) -> o x", o=1).to_broadcast(
                            (128, 512)))
                    for cb in range(CB):
                        oc = pb.tile([128, 512], BF16, tag="oc", bufs=2,
                                     name="oc")
                        nc.vector.tensor_mul(oc[:], o_sb[cb][:], rb[:])
                        final = pb.tile([128, 8, 2, WP, 2], BF16, tag="final",
                                        bufs=3, name="final")
                        up = oc.rearrange("c (h w) -> c h w", w=WP)[
                            :, :, :, None].to_broadcast((128, 8, WP, 2))
                        fvv = fv_sb[:, cb, ic * 16:(ic + 1) * 16, :].rearrange(
                            "c (h dy) (w dx) -> c h dy w dx", dy=2, dx=2)
                        nc.vector.tensor_add(final[:, :, 0], up, fvv[:, :, 0])
                        nc.vector.tensor_add(final[:, :, 1], up, fvv[:, :, 1])
                        nc.sync.dma_start(
                            out_d[cb * 128:(cb + 1) * 128,
                                  ic * 16:(ic + 1) * 16, :],
                            final.rearrange("c h dy w dx -> c (h dy) (w dx)"))

    nc.compile()
    return nc


_NC_CACHE = []
LAST_RESULT = []  # last BassKernelResults, for perf inspection by test.py


def kernel(**inputs) -> np.ndarray:
    fq = np.ascontiguousarray(
        np.asarray(inputs["feature_q"], dtype=np.float32).astype(
            ml_dtypes.bfloat16))
    fk = np.ascontiguousarray(
        np.asarray(inputs["feature_k"], dtype=np.float32).astype(
            ml_dtypes.bfloat16))
    fv = np.ascontiguousarray(
        np.asarray(inputs["feature_v"], dtype=np.float32).astype(
            ml_dtypes.bfloat16))
    wq = np.asarray(inputs["Wq"], dtype=np.float32)
    wk = np.asarray(inputs["Wk"], dtype=np.float32)
    wv = np.asarray(inputs["Wv"], dtype=np.float32)

    # weight layout prep (pure layout/scale folding, no heavy compute):
    # on-device pooling is a 2x2 *sum*; q,k each pick up 4x -> s is 16x,
    # folded into the on-device exp scale; v's 4x is folded into WvT here.
    wqt = np.ascontiguousarray(wq.T.astype(ml_dtypes.bfloat16))
    wkt = np.ascontiguousarray(wk.T.astype(ml_dtypes.bfloat16))
    wvt = np.ascontiguousarray(
        (wv.T * 0.25).astype(ml_dtypes.bfloat16))     # (C, C) [c_in, c_out]

    if not _NC_CACHE:
        _NC_CACHE.append(build_module())
    nc = _NC_CACHE[0]

    in_maps = [
        {
            "feature_q": fq[b],
            "feature_k": fk[b],
            "feature_v": fv[b],
            "WqT": wqt,
            "WkT": wkt,
            "WvT": wvt,
        }
        for b in range(B)
    ]
    res = run_bass_kernel_spmd(nc, in_maps, core_ids=list(range(B)))
    LAST_RESULT.clear()
    LAST_RESULT.append(res)
    out = np.stack([np.asarray(res.results[b]["out"]) for b in range(B)],
                   axis=0)
    return out.astype(np.float32)


if __name__ == "__main__":
    nc = build_module()
    print("module built + compiled OK")


# revision 10
# speedup vs baseline: 1.8218x; 1.0908x over previous
"""Trainium2 Bass kernel for nn_DCAM (dense transformer attention module).

Reference computation (per batch b):
  qp/kp/vp = avg_pool2d(feature_{q,k,v}, 2)            # (C=256, 64, 64)
  q = Wq @ qp, k = Wk @ kp  (M=32 channels)            # (32, N=4096)
  v = Wv @ vp                                          # (256, N)
  attn = softmax(q^T k, axis=-1)                       # (N, N)
  out[c, m] = sum_n v[c, n] attn[m, n]                 # (256, N)
  result = upsample_nearest(out, 2) + feature_v        # (256, 128, 128)

Sharding: data-parallel over batch B=8 across 8 NeuronCores (1 batch/core).

Per-core design (v2 — restructured from the hi/lo baseline):
  - All feature inputs are pre-cast to bf16 on the host; output is written
    bf16 and upcast on the host. Halves all HBM traffic.
  - q/k single bf16 (no hi/lo split): 1 S-term instead of 3. The 2e-2
    rel-err budget has ~7x slack over this.
  - The entire 2x2 sum-pooling of q/k is folded into the projection
    matmuls: 8 accumulating MMs per chunk with strided rhs APs
    (dy/dx slices of the raw 16x128 row block). No pooling DVE work at
    all on the q/k path.
  - v pooling stays a 2-step gpsimd add (from the resident fv copy);
    projection per j-block as before.
  - Phase order: fv+fk stream first (separate DMA queues) with V-pool/
    V-proj and K-proj interleaved; fq streams last and Phase B chases it
    per i-chunk, overlapping the attention with the tail of input DMA.
  - Phase B per jg: one [128,2048] S psum (4 j-blocks x 512 i), a single
    [128,2048] exp ACTIVATE (ACT does exp ONLY; all copies/evictions are
    on DVE), O-MMs per j-block/cb, and a bf16 DVE running sum for the
    softmax denominator (merged by a ones-matmul at i-chunk end).
  - softmax without max-subtraction (|s| <= ~15 fits f32/bf16 easily).
  - pooling is a 2x2 *sum*; scales fold into the exp scale (1/16) and
    into WvT (x0.25) on the host.
"""
import numpy as np
import ml_dtypes

import concourse.bass as bass
import concourse.mybir as mybir
import concourse.tile as tile
from concourse import bacc
from concourse.bass_utils import run_bass_kernel_spmd

F32 = mybir.dt.float32
BF16 = mybir.dt.bfloat16
AF = mybir.ActivationFunctionType

B = 8
C = 256
M = 32
H = W = 128
HP = WP = 64
N = HP * WP          # 4096
CB = C // 128        # 2 channel blocks
JB = N // 128        # 32 key blocks
JG = JB // 4         # 8 groups of 4 packed j-blocks
IC = N // 512        # 8 query chunks of 512


def build_module():
    nc = bacc.Bacc("TRN2", target_bir_lowering=False, debug=False)

    fq_d = nc.dram_tensor("feature_q", [C, H, W], BF16, kind="ExternalInput").ap()
    fk_d = nc.dram_tensor("feature_k", [C, H, W], BF16, kind="ExternalInput").ap()
    fv_d = nc.dram_tensor("feature_v", [C, H, W], BF16, kind="ExternalInput").ap()
    wqt_d = nc.dram_tensor("WqT", [C, M], BF16, kind="ExternalInput").ap()
    wkt_d = nc.dram_tensor("WkT", [C, M], BF16, kind="ExternalInput").ap()
    wvt_d = nc.dram_tensor("WvT", [C, C], BF16, kind="ExternalInput").ap()
    out_d = nc.dram_tensor("out", [C, H, W], BF16, kind="ExternalOutput").ap()

    with tile.TileContext(nc) as tc:
        with tc.tile_pool(name="const", bufs=1) as cpool, \
             tc.tile_pool(name="persist", bufs=1) as pp, \
             tc.tile_pool(name="ps", bufs=1, space="PSUM") as ps, \
             tc.tile_pool(name="dramb", bufs=2, space="DRAM") as dpool:
            # ---- constants ----
            wq_sb = cpool.tile([128, CB, M], BF16, name="wq")
            nc.sync.dma_start(wq_sb[:], wqt_d.rearrange("(b p) m -> p b m", p=128))
            wk_sb = cpool.tile([128, CB, M], BF16, name="wk")
            nc.sync.dma_start(wk_sb[:], wkt_d.rearrange("(b p) m -> p b m", p=128))
            wv_sb = cpool.tile([128, CB, C], BF16)
            nc.sync.dma_start(wv_sb[:], wvt_d.rearrange("(b p) c -> p b c", p=128))
            ones_b = cpool.tile([128, 1], BF16)
            nc.vector.memset(ones_b[:], 1.0)

            # ---- persistent tensors ----
            q4 = pp.tile([128, N], BF16)              # q replicated x4 groups
            k_all = pp.tile([128, JG, 128], BF16)     # [32*(jb%4)+m, jb//4, jf]
            vt_all = pp.tile([128, JB, C], BF16)      # vT[j, c] per j-block
            fv_sb = pp.tile([128, CB, H, W], BF16)    # resident residual copy

            # =========== Phase A: stream fv + fk, pool/project ===========
            # Queue assignment: fk on sync, fv on gpsimd, fq on the vector
            # queue. fq's tile rotation (bufs=3) self-throttles its stream
            # to stay just ahead of Phase B's per-i-chunk consumption, so
            # fk/fv get the HBM bandwidth first.
            def k_chunk(icn, feat, w_sb, is_q):
                xs = []
                for cb in range(CB):
                    x = pa.tile([128, 16, W], BF16, tag=f"x{'q' if is_q else 'k'}",
                                bufs=3, name="x")
                    eng = nc.gpsimd if is_q else nc.sync
                    eng.dma_start(
                        x[:], feat[cb * 128:(cb + 1) * 128,
                                   icn * 16:(icn + 1) * 16, :])
                    xs.append(x)
                pr_ps = ps.tile([128, 512], F32, tag="a", bufs=2,
                                name="pr_ps")[:M, :]
                mms = [(cb, dy, dx) for cb in range(CB)
                       for dy in range(2) for dx in range(2)]
                for mi, (cb, dy, dx) in enumerate(mms):
                    rhs = xs[cb].rearrange("c (h dy) (w dx) -> c h dy w dx",
                                           dy=2, dx=2)[:, :, dy, :, dx]
                    nc.tensor.matmul(pr_ps[:], w_sb[:, cb], rhs,
                                     start=(mi == 0), stop=(mi == len(mms) - 1),
                                     skip_group_check=True)
                cs = slice(icn * 512, (icn + 1) * 512)
                if is_q:
                    nc.vector.tensor_scalar_add(q4[0:32, cs], pr_ps[:], 0.0)
                    for g in range(1, 4):
                        nc.sync.dma_start(q4[g * 32:(g + 1) * 32, cs],
                                          q4[0:32, cs])
                else:
                    for t in range(4):
                        nc.vector.tensor_scalar_add(
                            k_all[t * 32:(t + 1) * 32, icn, :],
                            pr_ps[:, t * 128:(t + 1) * 128], 0.0)

            with tc.tile_pool(name="poolA", bufs=1) as pa:
                for slab in range(4):   # 32 raw rows -> 16 pooled rows each
                    r0 = slab * 32
                    for cb in range(CB):
                        nc.gpsimd.dma_start(
                            fv_sb[:, cb, r0:r0 + 32, :],
                            fv_d[cb * 128:(cb + 1) * 128, r0:r0 + 32, :])
                    vph = pa.tile([128, CB, 16, WP], BF16, tag="vph", bufs=2,
                                  name="vph")
                    for cb in range(CB):
                        src = fv_sb[:, cb, r0:r0 + 32, :].rearrange(
                            "c (h dy) (w dx) -> c h dy w dx", dy=2, dx=2)
                        rfv = pa.tile([128, 16, WP, 2], BF16, tag="rfv",
                                      bufs=2, name="rfv")
                        nc.vector.tensor_add(rfv[:], src[:, :, 0], src[:, :, 1])
                        nc.gpsimd.tensor_add(vph[:, cb], rfv[:, :, :, 0],
                                             rfv[:, :, :, 1])
                    for r2 in range(8):   # j-blocks in this slab
                        jb = slab * 8 + r2
                        vt_ps = ps.tile([128, 512], F32, tag="a",
                                        bufs=2, name="vt_ps")[:, :C]
                        nc.tensor.matmul(vt_ps[:],
                                         vph[:, 0, r2 * 2:r2 * 2 + 2, :],
                                         wv_sb[:, 0], start=True, stop=False)
                        nc.tensor.matmul(vt_ps[:],
                                         vph[:, 1, r2 * 2:r2 * 2 + 2, :],
                                         wv_sb[:, 1], start=False, stop=True)
                        nc.vector.tensor_scalar_add(vt_all[:, jb, :],
                                                    vt_ps[:], 0.0)
                    for icn in (slab * 2, slab * 2 + 1):
                        k_chunk(icn, fk_d, wk_sb, is_q=False)

                # ===== Phase B: attention, emitted inline per i-chunk =====
                # The PE queue is strict FIFO, so B(ic) is emitted right
                # after q-proj(ic): attention for chunk 0 starts as soon as
                # fk/fv + the first fq chunk have landed, and the tail of
                # the fq stream overlaps the attention steady state.
                # Software-pipelined by one jg: the O matmuls for jg-1 are
                # issued after exp(jg), so the PE streams O(jg-1) while ACT
                # computes exp(jg) - neither engine waits on the other, and
                # the PE stays HAM-warm.
                pb = pa

                def o_mms(ic, jg, p):
                    for t in range(4):
                        j = jg * 4 + t
                        pr = p[:, t * 512:(t + 1) * 512]
                        for cb in range(CB):
                            nc.tensor.matmul(
                                o_ps[cb][:],
                                vt_all[:, j, cb * 128:(cb + 1) * 128],
                                pr,
                                start=(j == 0), stop=(j == JB - 1),
                                skip_group_check=True)

                for ic in range(IC):
                    k_chunk(ic, fq_d, wq_sb, is_q=True)
                    i0 = ic * 512
                    lacc = pb.tile([128, 2048], BF16, tag="lacc", bufs=2,
                                   name="lacc")
                    o_ps = [ps.tile([128, 512], F32, tag=f"o{cb}", bufs=1,
                                    name=f"o{cb}_ps")
                            for cb in range(CB)]
                    p_prev = None
                    for jg in range(JG):
                        s_ps = ps.tile([128, 2048], F32, tag="s", bufs=1,
                                       name="s_ps")
                        for t in range(4):
                            gs = slice(t * 32, (t + 1) * 32)
                            nc.tensor.matmul(
                                s_ps[:, t * 512:(t + 1) * 512],
                                k_all[gs, jg, :], q4[gs, i0:i0 + 512],
                                start=True, stop=True,
                                tile_position=(t * 32, 0),
                                skip_group_check=True)
                        p = pb.tile([128, 2048], BF16, tag="p", bufs=4,
                                    name="p")
                        nc.scalar.activation(p[:], s_ps[:], AF.Exp,
                                             scale=0.0625)
                        if p_prev is not None:
                            o_mms(ic, jg - 1, p_prev)
                        if jg == 0:
                            nc.vector.tensor_scalar_add(lacc[:], p[:], 0.0)
                        else:
                            nc.vector.tensor_add(lacc[:], lacc[:], p[:])
                        p_prev = p
                    o_mms(ic, JG - 1, p_prev)
                    # ---- epilogue: fully async off the jg pipeline ----
                    # l first (its copy releases the s psum tag for the next
                    # i-chunk), then the o evictions release the o banks;
                    # 1/l + upsample+residual run from SBUF behind the scenes.
                    l_ps = ps.tile([128, 2048], F32, tag="s", bufs=1,
                                   name="l_ps")
                    for t in range(4):
                        nc.tensor.matmul(l_ps[:1, :512], ones_b[:],
                                         lacc[:, t * 512:(t + 1) * 512],
                                         start=(t == 0), stop=(t == 3),
                                         skip_group_check=True)
                    l_sb = pb.tile([1, 512], F32, tag="l_sb", bufs=2,
                                   name="l_sb")
                    nc.vector.tensor_scalar_add(l_sb[:], l_ps[:1, :512], 0.0)
                    o_sb = []
                    for cb in range(CB):
                        t_ = pb.tile([128, 512], F32, tag=f"osb{cb}", bufs=2,
                                     name=f"osb{cb}")
                        nc.vector.tensor_scalar_add(t_[:], o_ps[cb][:], 0.0)
                        o_sb.append(t_)
                    # 1/l on a (128, 4) transposed view (DRAM bounce), then
                    # broadcast back across partitions.
                    l_dr = dpool.tile([512], F32, tag="l_dr", bufs=2,
                                      name="l_dr")
                    nc.sync.dma_start(l_dr[:], l_sb[:])
                    lT = pb.tile([128, 4], F32, tag="lT", bufs=2, name="lT")
                    nc.sync.dma_start(lT[:], l_dr.rearrange("(p b) -> p b",
                                                            b=4))
                    rT = pb.tile([128, 4], F32, tag="rT", bufs=2, name="rT")
                    nc.vector.reciprocal(rT[:], lT[:])
                    r_dr = dpool.tile([512], F32, tag="r_dr", bufs=2,
                                      name="r_dr")
                    nc.sync.dma_start(r_dr.rearrange("(p b) -> p b", b=4),
                                      rT[:])
                    rb = pb.tile([128, 512], F32, tag="rb", bufs=2,
                                 name="rb")
                    nc.sync.dma_start(
                        rb[:],
                        r_dr.rearrange("(o x) -> o x", o=1).to_broadcast(
                            (128, 512)))
                    for cb in range(CB):
                        oc = pb.tile([128, 512], BF16, tag="oc", bufs=2,
                                     name="oc")
                        nc.vector.tensor_mul(oc[:], o_sb[cb][:], rb[:])
                        final = pb.tile([128, 8, 2, WP, 2], BF16, tag="final",
                                        bufs=3, name="final")
                        up = oc.rearrange("c (h w) -> c h w", w=WP)[
                            :, :, :, None].to_broadcast((128, 8, WP, 2))
                        fvv = fv_sb[:, cb, ic * 16:(ic + 1) * 16, :].rearrange(
                            "c (h dy) (w dx) -> c h dy w dx", dy=2, dx=2)
                        nc.vector.tensor_add(final[:, :, 0], up, fvv[:, :, 0])
                        nc.vector.tensor_add(final[:, :, 1], up, fvv[:, :, 1])
                        nc.sync.dma_start(
                            out_d[cb * 128:(cb + 1) * 128,
                                  ic * 16:(ic + 1) * 16, :],
                            final.rearrange("c h dy w dx -> c (h dy) (w dx)"))

    nc.compile()
    return nc


_NC_CACHE = []
LAST_RESULT = []  # last BassKernelResults, for perf inspection by test.py


def kernel(**inputs) -> np.ndarray:
    fq = np.ascontiguousarray(
        np.asarray(inputs["feature_q"], dtype=np.float32).astype(
            ml_dtypes.bfloat16))
    fk = np.ascontiguousarray(
        np.asarray(inputs["feature_k"], dtype=np.float32).astype(
            ml_dtypes.bfloat16))
    fv = np.ascontiguousarray(
        np.asarray(inputs["feature_v"], dtype=np.float32).astype(
            ml_dtypes.bfloat16))
    wq = np.asarray(inputs["Wq"], dtype=np.float32)
    wk = np.asarray(inputs["Wk"], dtype=np.float32)
    wv = np.asarray(inputs["Wv"], dtype=np.float32)

    # weight layout prep (pure layout/scale folding, no heavy compute):
    # on-device pooling is a 2x2 *sum*; q,k each pick up 4x -> s is 16x,
    # folded into the on-device exp scale; v's 4x is folded into WvT here.
    wqt = np.ascontiguousarray(wq.T.astype(ml_dtypes.bfloat16))
    wkt = np.ascontiguousarray(wk.T.astype(ml_dtypes.bfloat16))
    wvt = np.ascontiguousarray(
        (wv.T * 0.25).astype(ml_dtypes.bfloat16))     # (C, C) [c_in, c_out]

    if not _NC_CACHE:
        _NC_CACHE.append(build_module())
    nc = _NC_CACHE[0]

    in_maps = [
        {
            "feature_q": fq[b],
            "feature_k": fk[b],
            "feature_v": fv[b],
            "WqT": wqt,
            "WkT": wkt,
            "WvT": wvt,
        }
        for b in range(B)
    ]
    res = run_bass_kernel_spmd(nc, in_maps, core_ids=list(range(B)))
    LAST_RESULT.clear()
    LAST_RESULT.append(res)
    out = np.stack([np.asarray(res.results[b]["out"]) for b in range(B)],
                   axis=0)
    return out.astype(np.float32)


if __name__ == "__main__":
    nc = build_module()
    print("module built + compiled OK")
